# revision 1
# baseline (speedup 1.0000x reference)
"""Trainium2 Bass kernel for nn_Attention_36137854828870.

Multi-head causal attention with rotary embeddings:
  y = softmax((rope(x@wq) @ rope(x@wk)^T)/sqrt(hd) + causal) @ (x@wv) @ wo

Sharding (8 cores): data-parallel over batch (4) x tensor-parallel over
heads (2 groups of 8).  Core c handles batch c//2, head group c%2: it gets
column slices of wq/wk/wv and the matching row slice of wo, produces a
partial (S, D) output, and the host sums the two partials per batch
(cheaper than an in-kernel all-reduce at this size).

Per-core kernel (everything transposed so no on-chip transposes needed):
  1. Stream xT s-chunks; QT/KT = wq/wk-tile.T @ xT (d on partitions),
     V = xT-tile.T @ wv (s on partitions, with a ones column per head for
     the softmax denominator).  Per chunk: RoPE on QT/KT in a
     rope-friendly permutation (even dims of all heads in partition-tiles
     0-1, odd dims in 2-3, so pairs are lane-aligned), then DMA-shuffle
     the chunk to a head-contiguous layout in DRAM (QTb/KTb).
  2. Attention, q-block (1024) outer, head-pair inner, K/Q streamed back
     from DRAM: scoresT = K_h-tile.T @ QT (keys on partitions), exp on
     ScalarE with 1/sqrt(hd) folded into the activation scale (no
     max-subtraction: |scores| is tiny so fp32 exp is exact), causal mask
     on diagonal tiles via gpsimd.affine_select, P@V accumulated in PSUM
     (M=65: 64 head dims + denominator row).  The j-loop is software
     pipelined two deep (PV(j-2) is emitted after scores(j)/exp(j)) so
     neither PE nor ScalarE in-order-stalls on the other.  PSUM is
     evicted unnormalized (reciprocal of the denominator row lands in a
     small l-tile); the 1/l scale is applied afterwards, off the critical
     path, with a 0-stride broadcast DMA + in-place multiply.
  3. y = attnT-tile.T @ wo, DMA out.

All matmuls run as float32r (fp32 bits, reduced-precision multiply at
full PE rate); accumulation is fp32 in PSUM.  DMAs are spread across the
sync (loads) and gpsimd (shuffle/broadcast/store) queues so dispatch
doesn't serialize behind one sequencer.
"""

import sys

sys.path.insert(0, "/opt/trn_rl_repo")

import numpy as np

import concourse.bass as bass
import concourse.mybir as mybir
import concourse.tile as tile
from concourse import bacc
from concourse.bass_utils import run_bass_kernel_spmd

B, S, D = 4, 2048, 1024
H, HD = 16, 64
P = 128
NCORES = 8
HPC = H // 2          # heads per core
DG = HPC * HD         # 512: per-core head-group width
NKT = D // P          # 8 contraction tiles for projections
NDT = DG // P         # 4 partition-tiles of QT/KT
NSC = S // 512        # 4 s-chunks
NST = S // P          # 16 s(key)-tiles
QW = 1024             # attention q-block width
NQB = S // QW         # 2 q-blocks
F32 = mybir.dt.float32
F32R = mybir.dt.float32r

_PROGRAM = None


def _r(ap):
    return ap.bitcast(F32R)


def _build_program():
    nc = bacc.Bacc("TRN2", target_bir_lowering=False, debug=False)

    xT_d = nc.dram_tensor("xT", [D, S], F32R, kind="ExternalInput")
    wq_d = nc.dram_tensor("wq", [D, DG], F32R, kind="ExternalInput")
    wk_d = nc.dram_tensor("wk", [D, DG], F32R, kind="ExternalInput")
    wv_d = nc.dram_tensor("wv", [D, DG], F32R, kind="ExternalInput")
    wo_d = nc.dram_tensor("wo", [DG, D], F32R, kind="ExternalInput")
    cos_d = nc.dram_tensor("cost", [P, S], F32, kind="ExternalInput")
    sin_d = nc.dram_tensor("sint", [P, S], F32, kind="ExternalInput")
    y_d = nc.dram_tensor("y", [S, D], F32, kind="ExternalOutput")
    # permB (head-contiguous) Q/K staging in DRAM, viewed as 8 half-tiles
    # of 64 partitions: half-tile h holds head h's 64 dims.
    # staging lives in per-core output buffers: internal DRAM scratch can
    # alias across cores under this runtime (observed cross-core
    # clobbering).  Split per s-half so DRAM RAW deps (tracked per
    # tensor) don't make q-block 0's loads wait on chunks 2-3.
    qtb_h = [
        nc.dram_tensor(f"qtb_i{i}", [NDT, P, QW], F32R, kind="ExternalOutput")
        for i in range(2)
    ]
    ktb_h = [
        nc.dram_tensor(f"ktb_i{i}", [NDT, P, QW], F32R, kind="ExternalOutput")
        for i in range(2)
    ]

    xT_v = xT_d.ap().rearrange("(kt p) s -> p kt s", p=P)
    wq_v = wq_d.ap().rearrange("(kt p) m -> p kt m", p=P)
    wk_v = wk_d.ap().rearrange("(kt p) m -> p kt m", p=P)
    wv_v = wv_d.ap().rearrange("(kt p) m -> p kt m", p=P)
    wo_v = wo_d.ap().rearrange("(dt p) n -> p dt n", p=P)
    # (8 half-tiles, 64, S) views for the shuffle destinations
    qtb8 = [
        t.ap().rearrange("dtb (ht p) s -> (dtb ht) p s", ht=2) for t in qtb_h
    ]
    ktb8 = [
        t.ap().rearrange("dtb (ht p) s -> (dtb ht) p s", ht=2) for t in ktb_h
    ]

    with tile.TileContext(nc) as tc:
        with tc.tile_pool(name="vpool", bufs=1) as vpool:
            V = vpool.tile([P, NST, HPC, HD + 1], F32R, tag="V")
            ones = vpool.tile([P, NST * HPC], F32, tag="ones")
            nc.any.memset(ones[:], 1.0)
            nc.vector.tensor_copy(
                V[:, :, :, HD : HD + 1],
                ones[:].rearrange("p (a b) -> p a b", a=NST),
            )

            # ---- phase 1: projections + rope + shuffle, per s-chunk ----
            with tc.tile_pool(name="xw", bufs=2) as xw, \
                 tc.tile_pool(name="wres", bufs=1) as wres, \
                 tc.tile_pool(name="projout", bufs=1) as projout, \
                 tc.tile_pool(name="trig", bufs=1) as trig, \
                 tc.tile_pool(name="scr", bufs=1) as scr, \
                 tc.tile_pool(name="ps1", bufs=3, space="PSUM") as ps1:
                QT = projout.tile([P, NDT, S], F32, tag="QT")
                KT = projout.tile([P, NDT, S], F32, tag="KT")
                wqt = wres.tile([P, NKT, DG], F32R, tag="wq")
                wkt = wres.tile([P, NKT, DG], F32R, tag="wk")
                wvt = wres.tile([P, NKT, DG], F32R, tag="wv")
                cost = trig.tile([P, S], F32, tag="cos")
                sint = trig.tile([P, S], F32, tag="sin")
                # split loads, in consumption order, so the first matmuls
                # only wait on xc + wq
                xc0 = xw.tile([P, NKT, 512], F32R, tag="xc")
                # 3-way split of the first-chunk critical loads: sync, the
                # (idle) scalar HWDGE queue, and gpsimd each carry ~0.85MB
                nc.sync.dma_start(out=wkt[:, :, 0:P], in_=wk_v[:, :, 0:P])
                nc.sync.dma_start(out=xc0[:, 0:1, :], in_=xT_v[:, 0:1, 0:512])
                nc.scalar.dma_start(out=xc0[:, 1:4, :], in_=xT_v[:, 1:4, 0:512])
                nc.gpsimd.dma_start(out=xc0[:, 4:8, :], in_=xT_v[:, 4:8, 0:512])
                for dt in range(1, NDT):
                    nc.sync.dma_start(
                        out=wkt[:, :, dt * P : (dt + 1) * P],
                        in_=wk_v[:, :, dt * P : (dt + 1) * P],
                    )
                for dt in range(NDT):
                    nc.sync.dma_start(
                        out=wqt[:, :, dt * P : (dt + 1) * P],
                        in_=wq_v[:, :, dt * P : (dt + 1) * P],
                    )
                nc.sync.dma_start(out=wvt[:], in_=wv_v[:])
                nc.gpsimd.dma_start(out=cost[:], in_=cos_d.ap())
                nc.gpsimd.dma_start(out=sint[:], in_=sin_d.ap())

                for c in range(NSC):
                    csl = slice(c * 512, (c + 1) * 512)
                    if c == 0:
                        xc = xc0
                    else:
                        xc = xw.tile([P, NKT, 512], F32R, tag="xc")
                        nc.sync.dma_start(out=xc[:], in_=xT_v[:, :, csl])
                    def proj_mms(wt, out_t):
                        for dt in range(NDT):
                            psq = ps1.tile([P, 512], F32, tag="ps")
                            for kt in range(NKT):
                                nc.tensor.matmul(
                                    psq[:],
                                    wt[:, kt, dt * P : (dt + 1) * P],
                                    xc[:, kt, :],
                                    start=(kt == 0),
                                    stop=(kt == NKT - 1),
                                )
                            nc.scalar.copy(out_t[:, dt, csl], psq[:])
                    def rope_shuffle(out_t, dst8):
                        # rope (permA pairing: dt/dt+2 lane-aligned)
                        for dt in range(2):
                            a0 = out_t[:, dt, csl]
                            a1 = out_t[:, dt + 2, csl]
                            cc = cost[:, csl]
                            ss = sint[:, csl]
                            tt = scr.tile([P, 512], F32, tag="t")
                            uu = scr.tile([P, 512], F32, tag="u")
                            nc.vector.tensor_mul(tt[:], a0, ss)
                            nc.vector.tensor_mul(uu[:], a1, cc)
                            nc.vector.tensor_mul(a0, a0, cc)
                            nc.vector.tensor_mul(a1, a1, ss)
                            nc.vector.tensor_sub(a0, a0, a1)
                            nc.vector.tensor_add(a1, tt[:], uu[:])
                        # shuffle permA -> permB (head-contiguous) in DRAM.
                        # permA partition-tile dt holds 4 heads' 32-row
                        # strips; strip (h%4) of tile dt maps to half-tile
                        # h, rows [0,32) for evens (dt<2) or [32,64) for
                        # odds.  Order 0,2,1,3 finishes head-pair 0 first.
                        lsl = slice((c % 2) * 512, (c % 2) * 512 + 512)
                        for dt in (0, 2, 1, 3):
                            hbase = 4 * (dt % 2)
                            rlo = 32 * (dt // 2)
                            nc.gpsimd.dma_start(
                                out=dst8[c // 2][hbase : hbase + 4, rlo : rlo + 32, lsl],
                                in_=_r(out_t[:, dt, csl]),
                            )
                    proj_mms(wkt, KT)
                    rope_shuffle(KT, ktb8)
                    proj_mms(wqt, QT)
                    for st in range(4):
                        psv = ps1.tile([P, 512], F32, tag="ps")
                        for kt in range(NKT):
                            nc.tensor.matmul(
                                psv[:],
                                xc[:, kt, st * P : (st + 1) * P],
                                wvt[:, kt, :],
                                start=(kt == 0),
                                stop=(kt == NKT - 1),
                            )
                        nc.vector.tensor_copy(
                            V[:, c * 4 + st, :, 0:HD],
                            psv[:].rearrange("p (h d) -> p h d", h=HPC),
                        )
                    rope_shuffle(QT, qtb8)

            # ---- phase 2: attention ----
            with tc.tile_pool(name="atpool", bufs=1) as atpool:
                attnT = atpool.tile([P, NDT, S], F32R, tag="attnT")
                ltile = atpool.tile([P, 2, S], F32, tag="ltile")
                with tc.tile_pool(name="qkst", bufs=2) as qkst, \
                     tc.tile_pool(name="apsum", bufs=3, space="PSUM") as apsum, \
                     tc.tile_pool(name="opsum", bufs=1, space="PSUM") as opsum, \
                     tc.tile_pool(name="expool", bufs=6) as expool, \
                     tc.tile_pool(name="npool", bufs=2) as npool:
                    for qb in range(NQB):
                        kr = (qb + 1) * QW
                        njt = kr // P
                        qsl = slice(qb * QW, (qb + 1) * QW)
                        for dtb in range(NDT):  # head pair (2*dtb, 2*dtb+1)
                            kst = qkst.tile([P, S], F32R, tag="kst")
                            if qb == 0:
                                # gpsimd queue is still draining chunk-3
                                # shuffles (in-order dispatch) at this point
                                kq = (nc.sync, nc.sync, nc.sync, nc.sync)
                            else:
                                kq = (nc.sync, nc.gpsimd, nc.sync, nc.gpsimd)
                            for qi in range(kr // 512):
                                glo = qi * 512
                                kq[qi % 4].dma_start(
                                    out=kst[:, glo : glo + 512],
                                    in_=ktb_h[glo // QW].ap()[
                                        dtb, :, glo % QW : glo % QW + 512
                                    ],
                                )
                            qst = qkst.tile([P, QW], F32R, tag="qst")
                            nc.sync.dma_start(
                                out=qst[:, 0:512], in_=qtb_h[qb].ap()[dtb, :, 0:512]
                            )
                            (nc.sync if qb == 0 else nc.gpsimd).dma_start(
                                out=qst[:, 512:QW], in_=qtb_h[qb].ap()[dtb, :, 512:QW]
                            )
                            for hh in range(2):
                                pb = hh * 64
                                h = dtb * 2 + hh
                                pso = opsum.tile([P, QW], F32, tag="pso")

                                def emit_pv(j, pieces, ex):
                                    for lo, hi in pieces:
                                        nc.tensor.matmul(
                                            pso[0 : HD + 1, lo:hi],
                                            V[:, j, h, :],
                                            ex[:, lo:hi],
                                            start=(j == 0),
                                            stop=(j == njt - 1),
                                        )

                                pipe = []
                                for j in range(njt):
                                    diag = j >= njt - (QW // P)
                                    qlo = (j - (njt - QW // P)) * P if diag else 0
                                    pieces = (
                                        [(qlo, 512), (512, QW)]
                                        if qlo < 512
                                        else [(qlo, QW)]
                                    )
                                    pss = apsum.tile([P, QW], F32, tag="pss")
                                    for lo, hi in pieces:
                                        nc.tensor.matmul(
                                            pss[:, lo:hi],
                                            kst[pb : pb + 64, j * P : (j + 1) * P],
                                            qst[pb : pb + 64, lo:hi],
                                            start=True,
                                            stop=True,
                                        )
                                    ex = expool.tile([P, QW], F32R, tag="ex")
                                    nc.scalar.activation(
                                        ex[:, qlo:QW],
                                        pss[:, qlo:QW],
                                        mybir.ActivationFunctionType.Exp,
                                        scale=float(1.0 / np.sqrt(HD)),
                                    )
                                    if diag:
                                        nc.gpsimd.affine_select(
                                            out=ex[:, qlo : qlo + P],
                                            in_=ex[:, qlo : qlo + P],
                                            compare_op=mybir.AluOpType.is_ge,
                                            fill=0.0,
                                            base=0,
                                            pattern=[[1, P]],
                                            channel_multiplier=-1,
                                        )
                                    pipe.append((j, pieces, ex))
                                    if len(pipe) > 2:
                                        emit_pv(*pipe.pop(0))
                                for item in pipe:
                                    emit_pv(*item)

                                # unnormalized eviction; 1/l into the l-tile
                                nc.vector.reciprocal(
                                    ltile[(h % 4) * 32 : (h % 4) * 32 + 1, h // 4, qsl],
                                    pso[HD : HD + 1, :],
                                )
                                nc.vector.tensor_copy(
                                    attnT[pb : pb + HD, dtb, qsl], pso[0:HD, :]
                                )

                    # deferred normalization: attnT *= broadcast(1/l)
                    for qb in range(NQB):
                        qsl = slice(qb * QW, (qb + 1) * QW)
                        for h in range(HPC):
                            pb = (h % 2) * 64
                            dtb = h // 2
                            bc = npool.tile([P, QW], F32, tag="bc")
                            nc.gpsimd.dma_start(
                                out=bc[pb : pb + HD, :],
                                in_=ltile[
                                    (h % 4) * 32 : (h % 4) * 32 + 1, h // 4, qsl
                                ]
                                .unsqueeze(1)
                                .broadcast_to((1, HD, QW)),
                            )
                            nc.vector.tensor_mul(
                                attnT[pb : pb + HD, dtb, qsl],
                                attnT[pb : pb + HD, dtb, qsl],
                                bc[pb : pb + HD, :],
                            )

                    # ---- phase 3: output projection (shares the attention
                    # psum pool so there is no PSUM pool handoff) ----
                    with tc.tile_pool(name="wop", bufs=1) as wop, \
                         tc.tile_pool(name="ypool", bufs=3) as ypool:
                        wo_sb = wop.tile([P, NDT, D], F32R, tag="wo")
                        nc.sync.dma_start(out=wo_sb[:], in_=wo_v[:])
                        for qt16 in range(NST):
                            for nt in range(2):
                                psy = apsum.tile([P, QW], F32, tag="pss")
                                for dt in range(NDT):
                                    nc.tensor.matmul(
                                        psy[:, 0:512],
                                        attnT[:, dt, qt16 * P : (qt16 + 1) * P],
                                        wo_sb[:, dt, nt * 512 : (nt + 1) * 512],
                                        start=(dt == 0),
                                        stop=(dt == NDT - 1),
                                    )
                                yt = ypool.tile([P, 512], F32, tag="yt")
                                nc.scalar.copy(yt[:], psy[:, 0:512])
                                nc.sync.dma_start(
                                    out=y_d.ap()[
                                        qt16 * P : (qt16 + 1) * P,
                                        nt * 512 : (nt + 1) * 512,
                                    ],
                                    in_=yt[:],
                                )

    nc.compile()
    return nc


def _perm_a():
    """Column permutation for wq/wk: even head-dims of all heads first
    (head-major, 32 per head), then odd head-dims."""
    perm = np.empty(DG, dtype=np.int64)
    for n in range(DG):
        if n < DG // 2:
            h, i = n // 32, n % 32
            perm[n] = h * HD + 2 * i
        else:
            h, i = (n - DG // 2) // 32, (n - DG // 2) % 32
            perm[n] = h * HD + 2 * i + 1
    return perm


def kernel(**inputs):
    global _PROGRAM
    x = np.asarray(inputs["x"], dtype=np.float32)
    freqs_cos = np.asarray(inputs["freqs_cos"], dtype=np.float32)
    freqs_sin = np.asarray(inputs["freqs_sin"], dtype=np.float32)
    wq = np.asarray(inputs["wq"], dtype=np.float32)
    wk = np.asarray(inputs["wk"], dtype=np.float32)
    wv = np.asarray(inputs["wv"], dtype=np.float32)
    wo = np.asarray(inputs["wo"], dtype=np.float32)

    if _PROGRAM is None:
        _PROGRAM = _build_program()
    nc = _PROGRAM

    perm = _perm_a()
    # cos/sin tables: (S, HD//2) -> (128, S), row p holds cos[:, p % 32]
    cost = np.ascontiguousarray(np.tile(freqs_cos.T, (4, 1)))
    sint = np.ascontiguousarray(np.tile(freqs_sin.T, (4, 1)))

    in_maps = []
    for c in range(NCORES):
        b, g = c // 2, c % 2
        gsl = slice(g * DG, (g + 1) * DG)
        in_maps.append(
            {
                "xT": np.ascontiguousarray(x[b].T),
                "wq": np.ascontiguousarray(wq[:, gsl][:, perm]),
                "wk": np.ascontiguousarray(wk[:, gsl][:, perm]),
                "wv": np.ascontiguousarray(wv[:, gsl]),
                "wo": np.ascontiguousarray(wo[gsl, :]),
                "cost": cost,
                "sint": sint,
            }
        )

    res = run_bass_kernel_spmd(nc, in_maps, list(range(NCORES)))
    y = np.empty((B, S, D), dtype=np.float32)
    for b in range(B):
        y[b] = res.results[2 * b]["y"] + res.results[2 * b + 1]["y"]
    return y



# revision 10
# speedup vs baseline: 1.1079x; 1.1079x over previous
"""Trainium2 Bass kernel for nn_Attention_36137854828870 (v2).

Multi-head causal attention with rotary embeddings:
  y = softmax((rope(x@wq) @ rope(x@wk)^T)/sqrt(hd) + causal) @ (x@wv) @ wo

Sharding (8 cores): data-parallel over batch (4) x tensor-parallel over
heads (2 groups of 8); host sums the two partial y per batch.

v2 redesign vs v1 (317us):
  - fp16 end-to-end (tol is 2e-2; fp16 lands ~1e-3).  Halves DMA + SBUF,
    removes the fp32r small-N 4x matmul penalty.
  - q-blocks of 512 interleaved with projection chunks: attention on
    q-block b is emitted between projection chunks so PE never waits at a
    phase boundary; out-projection of earlier blocks fills PE while
    ScalarE catches up on exp late in the schedule.
  - exp batched 2-heads-at-a-time ([128, 2, 512] PSUM pairs): ~halves the
    ~450ns fixed cost per Activation instruction.
  - causal masking via a const triangular matmul accumulated into PSUM on
    the PE (cheap) instead of gpsimd.affine_select on ex.
  - softmax normalization: denominator row broadcast with one DMA per
    head and a single fp16 divide on DVE (replaces reciprocal +
    broadcast + multiply chain).
  - K/Q head-contiguous shuffle staged through DRAM in fp16 on HWDGE
    queues, loaded once into resident SBUF tiles (no per-qb reloads).
"""

import sys

sys.path.insert(0, "/opt/trn_rl_repo")

import numpy as np

import concourse.bass as bass
import concourse.mybir as mybir
import concourse.tile as tile
from concourse import bacc
from concourse.bass_utils import run_bass_kernel_spmd

B, S, D = 4, 2048, 1024
H, HD = 16, 64
P = 128
NCORES = 8
HPC = H // 2          # heads per core
DG = HPC * HD         # 512: per-core head-group width
NKT = D // P          # 8 contraction tiles for projections
NDT = DG // P         # 4 partition-tiles
CW = 512              # projection chunk width
NSC = S // CW         # 4 chunks
QW = 512              # attention q-block width
NQB = S // QW         # 4 q-blocks
F16 = mybir.dt.float16
F32 = mybir.dt.float32
NEG = -30000.0

_PROGRAM = None


def _build_program():
    nc = bacc.Bacc("TRN2", target_bir_lowering=False, debug=False)

    xT_d = nc.dram_tensor("xT", [D, S], F16, kind="ExternalInput")
    wq_d = nc.dram_tensor("wq", [D, DG], F16, kind="ExternalInput")
    wk_d = nc.dram_tensor("wk", [D, DG], F16, kind="ExternalInput")
    wv_d = nc.dram_tensor("wv", [D, DG], F16, kind="ExternalInput")
    wo_d = nc.dram_tensor("wo", [DG, D], F16, kind="ExternalInput")
    cos_d = nc.dram_tensor("cost", [P, S], F16, kind="ExternalInput")
    sin_d = nc.dram_tensor("sint", [P, S], F16, kind="ExternalInput")
    tri_d = nc.dram_tensor("tri", [P, P], F16, kind="ExternalInput")
    idn_d = nc.dram_tensor("idn", [P, P], F16, kind="ExternalInput")
    y_d = nc.dram_tensor("y", [S, D], F16, kind="ExternalOutput")
    # per-chunk head-contiguous staging (per-core output buffers: internal
    # DRAM scratch can alias across cores under this runtime)
    ktb_ds = [
        nc.dram_tensor(f"ktb{c}", [NDT, P, CW], F16, kind="ExternalOutput")
        for c in range(NSC)
    ]
    qtb_ds = [
        nc.dram_tensor(f"qtb{c}", [NDT, P, CW], F16, kind="ExternalOutput")
        for c in range(NSC)
    ]

    xT_v = xT_d.ap().rearrange("(kt p) s -> p kt s", p=P)
    wq_v = wq_d.ap().rearrange("(kt p) m -> p kt m", p=P)
    wk_v = wk_d.ap().rearrange("(kt p) m -> p kt m", p=P)
    wv_v = wv_d.ap().rearrange("(kt p) m -> p kt m", p=P)
    wo_v = wo_d.ap().rearrange("(dt p) n -> p dt n", p=P)

    with tile.TileContext(nc) as tc:
        with tc.tile_pool(name="res", bufs=1) as res, \
             tc.tile_pool(name="xw", bufs=2) as xw, \
             tc.tile_pool(name="qkc", bufs=2) as qkc, \
             tc.tile_pool(name="ropes", bufs=2) as ropes, \
             tc.tile_pool(name="expool", bufs=4) as expool, \
             tc.tile_pool(name="scrpool", bufs=4) as scrpool, \
             tc.tile_pool(name="bcpool", bufs=2) as bcpool, \
             tc.tile_pool(name="ytpool", bufs=3) as ytpool:
            # resident tiles
            V = res.tile([P, S // P, HPC, 66], F16, tag="V")
            KTb = res.tile([P, NDT, S], F16, tag="KTb")
            QTb = res.tile([P, NDT, S], F16, tag="QTb")
            attnT = res.tile([P, NDT, S], F16, tag="attnT")
            wqt = res.tile([P, NKT, DG], F16, tag="wq")
            wkt = res.tile([P, NKT, DG], F16, tag="wk")
            wvt = res.tile([P, NKT, DG], F16, tag="wv")
            wo_sb = res.tile([P, NDT, D], F16, tag="wo")
            cost = res.tile([P, S], F16, tag="cos")
            sint = res.tile([P, S], F16, tag="sin")
            tri_sb = res.tile([P, P], F16, tag="tri")
            idn_sb = res.tile([P, P], F16, tag="idn")
            onecol = res.tile([P, P], F16, tag="onecol")

            # ---- preloads, split across HWDGE queues; first K matmul only
            # needs wk dt0 + xc0 kt0 ----
            xc0 = xw.tile([P, NKT, CW], F16, tag="xc")
            nc.sync.dma_start(out=wkt[:, :, 0:P], in_=wk_v[:, :, 0:P])
            nc.sync.dma_start(out=xc0[:, 0:2, :], in_=xT_v[:, 0:2, 0:CW])
            nc.scalar.dma_start(out=xc0[:, 2:5, :], in_=xT_v[:, 2:5, 0:CW])
            nc.scalar.dma_start(out=xc0[:, 5:8, :], in_=xT_v[:, 5:8, 0:CW])
            for dt in range(1, NDT):
                nc.sync.dma_start(
                    out=wkt[:, :, dt * P : (dt + 1) * P],
                    in_=wk_v[:, :, dt * P : (dt + 1) * P],
                )
            for dt in range(NDT):
                nc.sync.dma_start(
                    out=wqt[:, :, dt * P : (dt + 1) * P],
                    in_=wq_v[:, :, dt * P : (dt + 1) * P],
                )
            nc.sync.dma_start(out=wvt[:], in_=wv_v[:])
            nc.scalar.dma_start(out=cost[:], in_=cos_d.ap())
            nc.scalar.dma_start(out=sint[:], in_=sin_d.ap())
            nc.scalar.dma_start(out=tri_sb[:], in_=tri_d.ap())
            nc.scalar.dma_start(out=idn_sb[:], in_=idn_d.ap())
            nc.scalar.dma_start(out=wo_sb[:], in_=wo_v[:])
            # ones column of V (softmax denominator rides the PV matmul)
            nc.any.memset(onecol[:], 1.0)
            nc.vector.tensor_copy(
                V[:, :, :, 64:65],
                onecol[:].rearrange("p (a b) -> p a b", a=S // P),
            )

            with tc.tile_pool(name="ps1", bufs=2, space="PSUM") as ps1, \
                 tc.tile_pool(name="pssA", bufs=2, space="PSUM") as pssA, \
                 tc.tile_pool(name="opool", bufs=2, space="PSUM") as opool:

                xcs = {0: xc0}

                def load_xc(c):
                    if c not in xcs:
                        xc = xw.tile([P, NKT, CW], F16, tag="xc")
                        nc.sync.dma_start(
                            out=xc[:], in_=xT_v[:, :, c * CW : (c + 1) * CW]
                        )
                        xcs[c] = xc
                    return xcs[c]

                def rope(tc_tile, csl):
                    for dt in range(2):
                        a0 = tc_tile[:, dt, :]
                        a1 = tc_tile[:, dt + 2, :]
                        cc = cost[:, csl]
                        ss = sint[:, csl]
                        t = ropes.tile([P, CW], F16, tag="rt")
                        u = ropes.tile([P, CW], F16, tag="ru")
                        nc.vector.tensor_mul(t[:], a0, ss)
                        nc.vector.tensor_mul(u[:], a1, cc)
                        nc.vector.tensor_mul(a0, a0, cc)
                        nc.vector.tensor_mul(a1, a1, ss)
                        nc.vector.tensor_sub(a0, a0, a1)
                        nc.vector.tensor_add(a1, t[:], u[:])

                def shuffle_load(tc_tile, stage_d, dest_sb, c, q):
                    # permA partition-strips -> head-contiguous halves in
                    # DRAM, then one load per dtb into the resident tile.
                    v8 = stage_d.ap().rearrange("dtb (h p) s -> (dtb h) p s", h=2)
                    for dt in range(NDT):
                        q.dma_start(
                            out=v8[
                                4 * (dt % 2) : 4 * (dt % 2) + 4,
                                32 * (dt // 2) : 32 * (dt // 2) + 32,
                                :,
                            ],
                            in_=tc_tile[:, dt, :],
                        )
                    for dtb in range(NDT):
                        q.dma_start(
                            out=dest_sb[:, dtb, c * CW : (c + 1) * CW],
                            in_=stage_d.ap()[dtb],
                        )

                def proj_seg_KQ(c, which):
                    csl = slice(c * CW, (c + 1) * CW)
                    xc = load_xc(c)
                    wt = wkt if which == "k" else wqt
                    dst = qkc.tile([P, NDT, CW], F16, tag=which + "c")
                    for dt in range(NDT):
                        ps = ps1.tile([P, CW], F32, tag="ps")
                        for kt in range(NKT):
                            nc.tensor.matmul(
                                ps[:],
                                wt[:, kt, dt * P : (dt + 1) * P],
                                xc[:, kt, :],
                                start=(kt == 0),
                                stop=(kt == NKT - 1),
                            )
                        if which == "k":
                            nc.scalar.copy(dst[:, dt, :], ps[:])
                        else:
                            nc.vector.tensor_copy(dst[:, dt, :], ps[:])
                    rope(dst, csl)
                    if which == "k":
                        shuffle_load(dst, ktb_ds[c], KTb, c, nc.sync)
                    else:
                        shuffle_load(dst, qtb_ds[c], QTb, c, nc.scalar)

                def proj_seg_V(c):
                    xc = load_xc(c)
                    for st in range(4):
                        ps = ps1.tile([P, CW], F32, tag="ps")
                        for kt in range(NKT):
                            nc.tensor.matmul(
                                ps[:],
                                xc[:, kt, st * P : (st + 1) * P],
                                wvt[:, kt, :],
                                start=(kt == 0),
                                stop=(kt == NKT - 1),
                            )
                        nc.vector.tensor_copy(
                            V[:, c * 4 + st, :, 0:64],
                            ps[:].rearrange("p (h d) -> p h d", h=HPC),
                        )

                def attn_hp(qb, hp, pssPool):
                    njt = 4 * (qb + 1)
                    q0 = qb * QW
                    pso = [
                        opool.tile([P, QW], F32, tag="pso", name=f"pso{qb}_{hp}_{_h}")
                        for _h in range(2)
                    ]
                    pipe = []

                    def emit_pv(j, qlo, ex):
                        for hh in range(2):
                            nc.tensor.matmul(
                                pso[hh][0:65, qlo:QW],
                                V[:, j, hp * 2 + hh, 0:65],
                                ex[:, hh, qlo:QW],
                                start=(j == 0),
                                stop=(j == njt - 1),
                            )

                    for j in range(njt):
                        diag = j >= njt - 4
                        qlo = (j - (njt - 4)) * P if diag else 0
                        pss = pssPool.tile([P, 2, QW], F32, tag="pss")
                        for hh in range(2):
                            nc.tensor.matmul(
                                pss[:, hh, qlo:QW],
                                KTb[64 * hh : 64 * hh + 64, hp, j * P : (j + 1) * P],
                                QTb[64 * hh : 64 * hh + 64, hp, q0 + qlo : q0 + QW],
                                start=True,
                                stop=not diag,
                            )
                            if diag:
                                nc.tensor.matmul(
                                    pss[:, hh, qlo : qlo + P],
                                    tri_sb[:],
                                    idn_sb[:],
                                    start=False,
                                    stop=True,
                                    skip_group_check=True,
                                )
                        ex = expool.tile([P, 2, QW], F16, tag="ex")
                        nc.scalar.activation(
                            ex[:, :, qlo:QW],
                            pss[:, :, qlo:QW],
                            mybir.ActivationFunctionType.Exp,
                            scale=float(1.0 / np.sqrt(HD)),
                        )
                        pipe.append((j, qlo, ex))
                        if len(pipe) > 2:
                            emit_pv(*pipe.pop(0))
                    for item in pipe:
                        emit_pv(*item)

                    # evict unnormalized (rows 0:64) + denominator (row 64),
                    # broadcast l, divide in fp16 on DVE
                    # normalize: recip of the denominator rows (f32, DVE
                    # reads PSUM), cast to fp16, broadcast-DMA down 64
                    # partitions, then all-fp16 multiplies on Pool.  Walrus
                    # requires equal start partitions on TensorTensor, so
                    # head hh's dims/bc/out all sit at partition base 64*hh.
                    qsl = slice(q0, q0 + QW)
                    lt = bcpool.tile([P, QW], F32, tag="lt")
                    ltf = bcpool.tile([P, QW], F16, tag="ltf")
                    scrs = []
                    for hh in range(2):
                        scr = scrpool.tile([P, QW], F16, tag="scr")
                        nc.vector.tensor_copy(
                            scr[64 * hh : 64 * hh + 64, :], pso[hh][0:64, :]
                        )
                        nc.vector.reciprocal(
                            lt[32 * hh : 32 * hh + 1, :], pso[hh][64:65, :]
                        )
                        scrs.append(scr)
                    nc.vector.tensor_copy(ltf[0:33, :], lt[0:33, :])
                    bc = bcpool.tile([P, QW], F16, tag="bc")
                    for hh in range(2):
                        nc.scalar.dma_start(
                            out=bc[64 * hh : 64 * hh + 64, :],
                            in_=ltf[32 * hh : 32 * hh + 1, :]
                            .unsqueeze(1)
                            .broadcast_to((1, 64, QW)),
                        )
                    for hh in range(2):
                        nc.gpsimd.tensor_mul(
                            attnT[64 * hh : 64 * hh + 64, hp, qsl],
                            scrs[hh][64 * hh : 64 * hh + 64, :],
                            bc[64 * hh : 64 * hh + 64, :],
                        )

                # ---- interleaved schedule, part A ----
                for c in (0, 1):
                    proj_seg_KQ(c, "k")
                    proj_seg_KQ(c, "q")
                    proj_seg_V(c)
                # qb0 interleaved with chunk 2
                proj_seg_KQ(2, "k")
                attn_hp(0, 0, pssA)
                proj_seg_KQ(2, "q")
                attn_hp(0, 1, pssA)
                proj_seg_V(2)
                attn_hp(0, 2, pssA)
                attn_hp(0, 3, pssA)
                # qb1 interleaved with chunk 3
                proj_seg_KQ(3, "k")
                attn_hp(1, 0, pssA)
                proj_seg_KQ(3, "q")
                attn_hp(1, 1, pssA)
                proj_seg_V(3)
                attn_hp(1, 2, pssA)
                attn_hp(1, 3, pssA)

            # ---- part B: qb2/qb3 with out-projection filler ----
            with tc.tile_pool(name="pssB", bufs=2, space="PSUM") as pssB, \
                 tc.tile_pool(name="opool2", bufs=2, space="PSUM") as opool2, \
                 tc.tile_pool(name="psy", bufs=2, space="PSUM") as psyp:

                def attn_hp2(qb, hp):
                    # same as attn_hp but uses part-B psum pools
                    njt = 4 * (qb + 1)
                    q0 = qb * QW
                    pso = [
                        opool2.tile([P, QW], F32, tag="pso", name=f"psoB{qb}_{hp}_{_h}")
                        for _h in range(2)
                    ]
                    pipe = []

                    def emit_pv(j, qlo, ex):
                        for hh in range(2):
                            nc.tensor.matmul(
                                pso[hh][0:65, qlo:QW],
                                V[:, j, hp * 2 + hh, 0:65],
                                ex[:, hh, qlo:QW],
                                start=(j == 0),
                                stop=(j == njt - 1),
                            )

                    for j in range(njt):
                        diag = j >= njt - 4
                        qlo = (j - (njt - 4)) * P if diag else 0
                        pss = pssB.tile([P, 2, QW], F32, tag="pss")
                        for hh in range(2):
                            nc.tensor.matmul(
                                pss[:, hh, qlo:QW],
                                KTb[64 * hh : 64 * hh + 64, hp, j * P : (j + 1) * P],
                                QTb[64 * hh : 64 * hh + 64, hp, q0 + qlo : q0 + QW],
                                start=True,
                                stop=not diag,
                            )
                            if diag:
                                nc.tensor.matmul(
                                    pss[:, hh, qlo : qlo + P],
                                    tri_sb[:],
                                    idn_sb[:],
                                    start=False,
                                    stop=True,
                                    skip_group_check=True,
                                )
                        ex = expool.tile([P, 2, QW], F16, tag="ex")
                        nc.scalar.activation(
                            ex[:, :, qlo:QW],
                            pss[:, :, qlo:QW],
                            mybir.ActivationFunctionType.Exp,
                            scale=float(1.0 / np.sqrt(HD)),
                        )
                        pipe.append((j, qlo, ex))
                        if len(pipe) > 2:
                            emit_pv(*pipe.pop(0))
                    for item in pipe:
                        emit_pv(*item)

                    # normalize: recip of the denominator rows (f32, DVE
                    # reads PSUM), cast to fp16, broadcast-DMA down 64
                    # partitions, then all-fp16 multiplies on Pool.  Walrus
                    # requires equal start partitions on TensorTensor, so
                    # head hh's dims/bc/out all sit at partition base 64*hh.
                    qsl = slice(q0, q0 + QW)
                    lt = bcpool.tile([P, QW], F32, tag="lt")
                    ltf = bcpool.tile([P, QW], F16, tag="ltf")
                    scrs = []
                    for hh in range(2):
                        scr = scrpool.tile([P, QW], F16, tag="scr")
                        nc.vector.tensor_copy(
                            scr[64 * hh : 64 * hh + 64, :], pso[hh][0:64, :]
                        )
                        nc.vector.reciprocal(
                            lt[32 * hh : 32 * hh + 1, :], pso[hh][64:65, :]
                        )
                        scrs.append(scr)
                    nc.vector.tensor_copy(ltf[0:33, :], lt[0:33, :])
                    bc = bcpool.tile([P, QW], F16, tag="bc")
                    for hh in range(2):
                        nc.scalar.dma_start(
                            out=bc[64 * hh : 64 * hh + 64, :],
                            in_=ltf[32 * hh : 32 * hh + 1, :]
                            .unsqueeze(1)
                            .broadcast_to((1, 64, QW)),
                        )
                    for hh in range(2):
                        nc.gpsimd.tensor_mul(
                            attnT[64 * hh : 64 * hh + 64, hp, qsl],
                            scrs[hh][64 * hh : 64 * hh + 64, :],
                            bc[64 * hh : 64 * hh + 64, :],
                        )

                def op_group(qb, g):
                    qt = qb * 4 + g // 2
                    nt = g % 2
                    psy = psyp.tile([P, QW], F32, tag="psy")
                    for dt in range(NDT):
                        nc.tensor.matmul(
                            psy[:],
                            attnT[:, dt, qt * P : (qt + 1) * P],
                            wo_sb[:, dt, nt * 512 : (nt + 1) * 512],
                            start=(dt == 0),
                            stop=(dt == NDT - 1),
                        )
                    yt = ytpool.tile([P, 512], F16, tag="yt")
                    if g % 2 == 0:
                        nc.vector.tensor_copy(yt[:], psy[:])
                    else:
                        nc.scalar.copy(yt[:], psy[:])
                    nc.sync.dma_start(
                        out=y_d.ap()[
                            qt * P : (qt + 1) * P, nt * 512 : (nt + 1) * 512
                        ],
                        in_=yt[:],
                    )

                # qb2 with out-proj of qb0 as PE filler
                attn_hp2(2, 0)
                op_group(0, 0); op_group(0, 1)
                attn_hp2(2, 1)
                op_group(0, 2); op_group(0, 3)
                attn_hp2(2, 2)
                op_group(0, 4); op_group(0, 5)
                attn_hp2(2, 3)
                op_group(0, 6); op_group(0, 7)
                # qb3 with out-proj of qb1/qb2 as filler
                attn_hp2(3, 0)
                for g in range(4):
                    op_group(1, g)
                attn_hp2(3, 1)
                for g in range(4, 8):
                    op_group(1, g)
                attn_hp2(3, 2)
                for g in range(4):
                    op_group(2, g)
                attn_hp2(3, 3)
                for g in range(4, 8):
                    op_group(2, g)
                for g in range(8):
                    op_group(3, g)

    nc.compile()
    return nc


def _perm_a():
    """Column permutation for wq/wk: even head-dims of all heads first
    (head-major, 32 per head), then odd head-dims."""
    perm = np.empty(DG, dtype=np.int64)
    for n in range(DG):
        if n < DG // 2:
            h, i = n // 32, n % 32
            perm[n] = h * HD + 2 * i
        else:
            h, i = (n - DG // 2) // 32, (n - DG // 2) % 32
            perm[n] = h * HD + 2 * i + 1
    return perm


def kernel(**inputs):
    global _PROGRAM
    x = np.asarray(inputs["x"], dtype=np.float32)
    freqs_cos = np.asarray(inputs["freqs_cos"], dtype=np.float32)
    freqs_sin = np.asarray(inputs["freqs_sin"], dtype=np.float32)
    wq = np.asarray(inputs["wq"], dtype=np.float32)
    wk = np.asarray(inputs["wk"], dtype=np.float32)
    wv = np.asarray(inputs["wv"], dtype=np.float32)
    wo = np.asarray(inputs["wo"], dtype=np.float32)

    if _PROGRAM is None:
        _PROGRAM = _build_program()
    nc = _PROGRAM

    perm = _perm_a()
    cost = np.ascontiguousarray(np.tile(freqs_cos.T, (4, 1))).astype(np.float16)
    sint = np.ascontiguousarray(np.tile(freqs_sin.T, (4, 1))).astype(np.float16)
    col = np.arange(P)[None, :]
    row = np.arange(P)[:, None]
    tri = np.where(col > row, np.float16(NEG), np.float16(0.0)).astype(np.float16)
    idn = np.eye(P, dtype=np.float16)

    in_maps = []
    for c in range(NCORES):
        b, g = c // 2, c % 2
        gsl = slice(g * DG, (g + 1) * DG)
        in_maps.append(
            {
                "xT": np.ascontiguousarray(x[b].T).astype(np.float16),
                "wq": np.ascontiguousarray(wq[:, gsl][:, perm]).astype(np.float16),
                "wk": np.ascontiguousarray(wk[:, gsl][:, perm]).astype(np.float16),
                "wv": np.ascontiguousarray(wv[:, gsl]).astype(np.float16),
                "wo": np.ascontiguousarray(wo[gsl, :]).astype(np.float16),
                "cost": cost,
                "sint": sint,
                "tri": tri,
                "idn": idn,
            }
        )

    res = run_bass_kernel_spmd(nc, in_maps, list(range(NCORES)))
    y = np.empty((B, S, D), dtype=np.float32)
    for b in range(B):
        y[b] = res.results[2 * b]["y"].astype(np.float32) + res.results[
            2 * b + 1
        ]["y"].astype(np.float32)
    return y
